# revision 18
# baseline (speedup 1.0000x reference)
"""Trainium2 Bass kernel for nn_Decoder_36636071035490.

Reference computes, for workers i and task/edge (j,l):
    z = worker_feature @ W            # [2000, 1]
    p1 = sigmoid(z + b)
    p2 = (1 - p1) / 9
    P[i, j, l] = p1_i^tau_jl * p2_i^(1 - tau_jl)      # [2000, 5000, 10] f32

Identity used on device (exact in exact arithmetic):
    P[i, f] = exp(a_i * tau_f + c_i)
    a_i = (z_i + b) + ln 9            # since logit(sigmoid(x)) = x
    c_i = -ln(1 + exp(z_i + b)) - ln 9

Sharding: by output columns (task*edge flattened, 50000 -> 8 x 6250); every
core computes the cheap per-worker scalars a/c for all 2000 workers
(replicated matvec) and produces the full-height [2000, 6250] slab of P,
which is contiguous in its own output tensor. tau for the core's column
slice is passed pre-replicated across the 128 SBUF partitions ([128, 6250],
3.2 MB) so it loads as one balanced 16-engine DMA. The heavy math is a
single ScalarE ACTIVATE per 128-worker tile: out[p,f] = Exp(a_p*tau[f]+c_p)
via the activation's per-partition scale/bias operands. No PE, no PSUM.
All DMAs are full-128-partition (the SDMA splitter only uses all 16 engines
then); worker tile 15 overlaps tile 14 (rows 1872..1919 stored twice with
identical data) to keep 2000 = 15*128 + 80 full-width.
"""

import numpy as np

WORKERS = 2000
TASKS = 5000
ET = 10
AB = 64
NCORES = 8
F = TASKS * ET  # 50000 output cols
FS = F // NCORES  # 6250 cols per core
LN9 = float(np.log(9.0))

# worker tiles: 15 aligned tiles + one overlapping tail tile
_WSTARTS = [128 * t for t in range(15)] + [WORKERS - 128]

_CACHE = {}


def _build_nc():
    import concourse.bass as bass
    import concourse.mybir as mybir
    from concourse import bacc
    from concourse.tile import TileContext
    from contextlib import ExitStack

    f32 = mybir.dt.float32
    AF = mybir.ActivationFunctionType
    OP = mybir.AluOpType

    nc = bacc.Bacc("TRN2")
    wk = nc.dram_tensor("wk", [WORKERS, AB], f32, kind="ExternalInput")
    # this core's tau column slice, pre-replicated across 128 partitions
    tfb = nc.dram_tensor("tfb", [128, FS], f32, kind="ExternalInput")
    Wd = nc.dram_tensor("W", [AB, 1], f32, kind="ExternalInput")
    bd = nc.dram_tensor("b", [1], f32, kind="ExternalInput")
    out = nc.dram_tensor("out", [WORKERS, FS], f32, kind="ExternalOutput")

    NT = len(_WSTARTS)

    with TileContext(nc) as tc, ExitStack() as ctx:
        const = ctx.enter_context(tc.tile_pool(name="const", bufs=1))
        stage_p = ctx.enter_context(tc.tile_pool(name="stagep", bufs=2))

        # ---- per-worker scalars a (ACT scale) and c (ACT bias), all tiles
        Wb = const.tile([128, AB], f32, name="Wb")
        nc.sync.dma_start(
            out=Wb, in_=Wd[:].rearrange("a b -> b a").to_broadcast((128, AB))
        )
        bcol = const.tile([128, 1], f32, name="bcol")
        nc.sync.dma_start(out=bcol, in_=bd[:].to_broadcast((128, 1)))

        # ---- per-worker scalars for all 16 tiles at once: lane p of column t
        # is worker 128*t+p (tail tile 15 starts at 1872 instead of 1920).
        # Batching keeps the whole setup at 2 activations -> <=2 table loads.
        wka = const.tile([128, NT, AB], f32, name="wka")
        src15 = wk[0 : 15 * 128, :].rearrange("(t p) a -> p t a", p=128)
        nc.sync.dma_start(out=wka[:, 0:15, :], in_=src15)
        srct = wk[WORKERS - 128 : WORKERS, :].rearrange("(o p) a -> p o a", o=1)
        nc.sync.dma_start(out=wka[:, 15:16, :], in_=srct)
        # multiply by W broadcast along the tile dim (step-0 free dim read)
        Wb16 = bass.AP(
            tensor=Wb.tensor, offset=Wb.offset, ap=[list(Wb.ap[0]), [0, NT], [1, AB]]
        )
        proda = const.tile([128, NT, AB], f32, name="proda")
        nc.vector.tensor_mul(proda, wka, Wb16)
        zall = const.tile([128, NT], f32, name="zall")
        nc.vector.reduce_sum(
            out=zall.rearrange("p (t o) -> p t o", o=1),
            in_=proda,
            axis=mybir.AxisListType.X,
        )
        aall = const.tile([128, NT], f32, name="aall")
        nc.vector.tensor_scalar(
            out=aall, in0=zall, scalar1=bcol, scalar2=LN9, op0=OP.add, op1=OP.add
        )
        eall = const.tile([128, NT], f32, name="eall")
        nc.scalar.activation(out=eall, in_=zall, func=AF.Exp, bias=bcol, scale=1.0)
        lall = const.tile([128, NT], f32, name="lall")
        nc.scalar.activation(out=lall, in_=eall, func=AF.Ln, bias=1.0, scale=1.0)
        call = const.tile([128, NT], f32, name="call")
        nc.vector.tensor_scalar(
            out=call, in0=lall, scalar1=-1.0, scalar2=-LN9, op0=OP.mult, op1=OP.add
        )
        acol = [aall[:, t : t + 1] for t in range(NT)]
        ccol = [call[:, t : t + 1] for t in range(NT)]

        # ---- tau slice for this core (loaded once, balanced 16-engine DMA,
        # issued from the otherwise-idle GpSimd ring)
        taub = const.tile([128, FS], f32, name="taub")
        nc.gpsimd.dma_start(out=taub, in_=tfb[:])

        # ---- main loop: two worker tiles per staged store (fewer DMAs and
        # semaphore hops on the store ring); store is one 3D-AP DMA covering
        # 256 output rows
        for p in range(NT // 2):
            t0, t1 = 2 * p, 2 * p + 1
            stg = stage_p.tile([128, 2, FS], f32, name="stg", tag="stg")
            nc.scalar.activation(
                out=stg[:, 0, :],
                in_=taub,
                func=AF.Exp,
                bias=ccol[t0],
                scale=acol[t0],
            )
            nc.scalar.activation(
                out=stg[:, 1, :],
                in_=taub,
                func=AF.Exp,
                bias=ccol[t1],
                scale=acol[t1],
            )
            w0, w1 = _WSTARTS[t0], _WSTARTS[t1]
            if w1 == w0 + 128:
                dst = out[w0 : w0 + 256, :].rearrange("(c w) f -> w c f", c=2)
            else:
                # last pair overlaps (1792, 1872); build the two row-blocks
                # as an explicit strided AP on the row dimension
                dst = bass.AP(
                    tensor=out[:].tensor,
                    offset=w0 * FS,
                    ap=[[FS, 128], [(w1 - w0) * FS, 2], [1, FS]],
                )
            nc.sync.dma_start(out=dst, in_=stg)

    nc.compile()
    return nc


def _get_nc():
    if "nc" not in _CACHE:
        _CACHE["nc"] = _build_nc()
    return _CACHE["nc"]


def _make_in_maps(inputs_arr, W, b):
    wk = np.ascontiguousarray(inputs_arr[:WORKERS, :AB], dtype=np.float32)
    tau_flat = np.ascontiguousarray(
        inputs_arr[WORKERS:, :ET], dtype=np.float32
    ).reshape(F)
    W = np.ascontiguousarray(W, dtype=np.float32)
    b = np.ascontiguousarray(b, dtype=np.float32)
    maps = []
    for c in range(NCORES):
        tfb = np.ascontiguousarray(
            np.broadcast_to(tau_flat[c * FS : (c + 1) * FS], (128, FS))
        )
        maps.append({"wk": wk, "tfb": tfb, "W": W, "b": b})
    return maps


def _run(inputs_arr, W, b, **kwargs):
    from concourse import bass_utils

    nc = _get_nc()
    in_maps = _make_in_maps(inputs_arr, W, b)
    return bass_utils.run_bass_kernel_spmd(
        nc, in_maps, core_ids=list(range(NCORES)), **kwargs
    )


def kernel(inputs, W, b):
    inputs_arr = np.asarray(inputs, dtype=np.float32)
    res = _run(inputs_arr, np.asarray(W), np.asarray(b))
    out = np.concatenate([r["out"] for r in res.results], axis=1)
    return out.reshape(WORKERS, TASKS, ET)


# revision 21
# speedup vs baseline: 1.1677x; 1.1677x over previous
"""Trainium2 Bass kernel for nn_Decoder_36636071035490.

Reference computes, for workers i and task/edge (j,l):
    z = worker_feature @ W            # [2000, 1]
    p1 = sigmoid(z + b)
    p2 = (1 - p1) / 9
    P[i, j, l] = p1_i^tau_jl * p2_i^(1 - tau_jl)      # [2000, 5000, 10] f32

Identity used on device (exact in exact arithmetic):
    P[i, f] = exp(a_i * tau_f + c_i)
    a_i = (z_i + b) + ln 9            # since logit(sigmoid(x)) = x
    c_i = -ln(1 + exp(z_i + b)) - ln 9

Sharding: by output columns (task*edge flattened, 50000 -> 8 x 6250); every
core computes the cheap per-worker scalars a/c for all 2000 workers
(replicated matvec) and produces the full-height [2000, 6250] slab of P,
which is contiguous in its own output tensor. tau for the core's column
slice is passed pre-replicated across the 128 SBUF partitions ([128, 6250],
3.2 MB) so it loads as one balanced 16-engine DMA. The heavy math is a
single ScalarE ACTIVATE per 128-worker tile: out[p,f] = Exp(a_p*tau[f]+c_p)
via the activation's per-partition scale/bias operands. No PE, no PSUM.
All DMAs are full-128-partition (the SDMA splitter only uses all 16 engines
then); worker tile 15 overlaps tile 14 (rows 1872..1919 stored twice with
identical data) to keep 2000 = 15*128 + 80 full-width.
"""

import numpy as np

WORKERS = 2000
TASKS = 5000
ET = 10
AB = 64
NCORES = 8
F = TASKS * ET  # 50000 output cols
FS = F // NCORES  # 6250 cols per core
LN9 = float(np.log(9.0))

# worker tiles: 15 aligned tiles + one overlapping tail tile
_WSTARTS = [128 * t for t in range(15)] + [WORKERS - 128]

_CACHE = {}


def _build_nc():
    import concourse.bass as bass
    import concourse.mybir as mybir
    from concourse import bacc
    from concourse.tile import TileContext
    from contextlib import ExitStack

    f32 = mybir.dt.float32
    AF = mybir.ActivationFunctionType
    OP = mybir.AluOpType

    nc = bacc.Bacc("TRN2")
    wk = nc.dram_tensor("wk", [WORKERS, AB], f32, kind="ExternalInput")
    # this core's tau column slice, pre-replicated across 128 partitions
    tfb = nc.dram_tensor("tfb", [128, FS], f32, kind="ExternalInput")
    Wd = nc.dram_tensor("W", [AB, 1], f32, kind="ExternalInput")
    bd = nc.dram_tensor("b", [1], f32, kind="ExternalInput")
    out = nc.dram_tensor("out", [WORKERS, FS], f32, kind="ExternalOutput")

    NT = len(_WSTARTS)

    with TileContext(nc) as tc, ExitStack() as ctx:
        const = ctx.enter_context(tc.tile_pool(name="const", bufs=1))
        stage_p = ctx.enter_context(tc.tile_pool(name="stagep", bufs=3))

        # ---- per-worker scalars a (ACT scale) and c (ACT bias), all tiles
        Wb = const.tile([128, AB], f32, name="Wb")
        nc.sync.dma_start(
            out=Wb, in_=Wd[:].rearrange("a b -> b a").to_broadcast((128, AB))
        )
        bcol = const.tile([128, 1], f32, name="bcol")
        nc.sync.dma_start(out=bcol, in_=bd[:].to_broadcast((128, 1)))

        # ---- tau slice for this core (loaded once, balanced 16-engine DMA)
        taub = const.tile([128, FS], f32, name="taub")
        nc.scalar.dma_start(out=taub, in_=tfb[:])

        # ---- per-worker scalars in 4 batches of 4 tiles: lane p of column t
        # is worker 128*t+p (tail tile 15 starts at 1872 instead of 1920).
        # Batch 0 unblocks the first main ACT pair early; batching keeps the
        # activation count (and hence ACT table reloads) small.
        NB, TB = 4, NT // 4
        acol, ccol = [None] * NT, [None] * NT
        for bi in range(NB):
            tlo = bi * TB
            wka = const.tile([128, TB, AB], f32, name=f"wka{bi}", tag="wka", bufs=2)
            if bi < NB - 1:
                srcb = wk[tlo * 128 : (tlo + TB) * 128, :].rearrange(
                    "(t p) a -> p t a", p=128
                )
                nc.sync.dma_start(out=wka, in_=srcb)
            else:
                srcb = wk[tlo * 128 : (tlo + TB - 1) * 128, :].rearrange(
                    "(t p) a -> p t a", p=128
                )
                nc.sync.dma_start(out=wka[:, 0 : TB - 1, :], in_=srcb)
                srct = wk[WORKERS - 128 : WORKERS, :].rearrange(
                    "(o p) a -> p o a", o=1
                )
                nc.sync.dma_start(out=wka[:, TB - 1 : TB, :], in_=srct)
            WbT = bass.AP(
                tensor=Wb.tensor,
                offset=Wb.offset,
                ap=[list(Wb.ap[0]), [0, TB], [1, AB]],
            )
            proda = const.tile(
                [128, TB, AB], f32, name=f"proda{bi}", tag="proda", bufs=2
            )
            nc.vector.tensor_mul(proda, wka, WbT)
            zb_ = const.tile([128, TB], f32, name=f"zb{bi}", tag="zb", bufs=2)
            nc.vector.reduce_sum(
                out=zb_.rearrange("p (t o) -> p t o", o=1),
                in_=proda,
                axis=mybir.AxisListType.X,
            )
            ab_ = const.tile([128, TB], f32, name=f"ab{bi}")
            nc.vector.tensor_scalar(
                out=ab_, in0=zb_, scalar1=bcol, scalar2=LN9, op0=OP.add, op1=OP.add
            )
            eb_ = const.tile([128, TB], f32, name=f"eb{bi}", tag="eb", bufs=2)
            nc.scalar.activation(out=eb_, in_=zb_, func=AF.Exp, bias=bcol, scale=1.0)
            lb_ = const.tile([128, TB], f32, name=f"lb{bi}", tag="lb", bufs=2)
            nc.scalar.activation(out=lb_, in_=eb_, func=AF.Ln, bias=1.0, scale=1.0)
            cb_ = const.tile([128, TB], f32, name=f"cb{bi}")
            nc.vector.tensor_scalar(
                out=cb_, in0=lb_, scalar1=-1.0, scalar2=-LN9, op0=OP.mult, op1=OP.add
            )
            for j in range(TB):
                acol[tlo + j] = ab_[:, j : j + 1]
                ccol[tlo + j] = cb_[:, j : j + 1]

        # ---- main loop: two worker tiles per staged store (fewer DMAs and
        # semaphore hops on the store ring); store is one 3D-AP DMA covering
        # 256 output rows
        for p in range(NT // 2):
            t0, t1 = 2 * p, 2 * p + 1
            stg = stage_p.tile([128, 2, FS], f32, name="stg", tag="stg")
            nc.scalar.activation(
                out=stg[:, 0, :],
                in_=taub,
                func=AF.Exp,
                bias=ccol[t0],
                scale=acol[t0],
            )
            nc.scalar.activation(
                out=stg[:, 1, :],
                in_=taub,
                func=AF.Exp,
                bias=ccol[t1],
                scale=acol[t1],
            )
            w0, w1 = _WSTARTS[t0], _WSTARTS[t1]
            if w1 == w0 + 128:
                dst = out[w0 : w0 + 256, :].rearrange("(c w) f -> w c f", c=2)
            else:
                # last pair overlaps (1792, 1872); build the two row-blocks
                # as an explicit strided AP on the row dimension
                dst = bass.AP(
                    tensor=out[:].tensor,
                    offset=w0 * FS,
                    ap=[[FS, 128], [(w1 - w0) * FS, 2], [1, FS]],
                )
            nc.sync.dma_start(out=dst, in_=stg)

    nc.compile()
    return nc


def _get_nc():
    if "nc" not in _CACHE:
        _CACHE["nc"] = _build_nc()
    return _CACHE["nc"]


def _make_in_maps(inputs_arr, W, b):
    wk = np.ascontiguousarray(inputs_arr[:WORKERS, :AB], dtype=np.float32)
    tau_flat = np.ascontiguousarray(
        inputs_arr[WORKERS:, :ET], dtype=np.float32
    ).reshape(F)
    W = np.ascontiguousarray(W, dtype=np.float32)
    b = np.ascontiguousarray(b, dtype=np.float32)
    maps = []
    for c in range(NCORES):
        tfb = np.ascontiguousarray(
            np.broadcast_to(tau_flat[c * FS : (c + 1) * FS], (128, FS))
        )
        maps.append({"wk": wk, "tfb": tfb, "W": W, "b": b})
    return maps


def _run(inputs_arr, W, b, **kwargs):
    from concourse import bass_utils

    nc = _get_nc()
    in_maps = _make_in_maps(inputs_arr, W, b)
    return bass_utils.run_bass_kernel_spmd(
        nc, in_maps, core_ids=list(range(NCORES)), **kwargs
    )


def kernel(inputs, W, b):
    inputs_arr = np.asarray(inputs, dtype=np.float32)
    res = _run(inputs_arr, np.asarray(W), np.asarray(b))
    out = np.concatenate([r["out"] for r in res.results], axis=1)
    return out.reshape(WORKERS, TASKS, ET)


# revision 23
# speedup vs baseline: 1.2104x; 1.0366x over previous
"""Trainium2 Bass kernel for nn_Decoder_36636071035490.

Reference computes, for workers i and task/edge (j,l):
    z = worker_feature @ W            # [2000, 1]
    p1 = sigmoid(z + b)
    p2 = (1 - p1) / 9
    P[i, j, l] = p1_i^tau_jl * p2_i^(1 - tau_jl)      # [2000, 5000, 10] f32

Identity used on device (exact in exact arithmetic):
    P[i, f] = exp(a_i * tau_f + c_i)
    a_i = (z_i + b) + ln 9            # since logit(sigmoid(x)) = x
    c_i = -ln(1 + exp(z_i + b)) - ln 9

Sharding: by output columns (task*edge flattened, 50000 -> 8 x 6250); every
core computes the cheap per-worker scalars a/c for all 2000 workers
(replicated matvec) and produces the full-height [2000, 6250] slab of P,
which is contiguous in its own output tensor. tau for the core's column
slice is passed pre-replicated across the 128 SBUF partitions ([128, 6250],
3.2 MB) so it loads as one balanced 16-engine DMA. The heavy math is a
single ScalarE ACTIVATE per 128-worker tile: out[p,f] = Exp(a_p*tau[f]+c_p)
via the activation's per-partition scale/bias operands. No PE, no PSUM.
All DMAs are full-128-partition (the SDMA splitter only uses all 16 engines
then); worker tile 15 overlaps tile 14 (rows 1872..1919 stored twice with
identical data) to keep 2000 = 15*128 + 80 full-width.
"""

import numpy as np

WORKERS = 2000
TASKS = 5000
ET = 10
AB = 64
NCORES = 8
F = TASKS * ET  # 50000 output cols
FS = F // NCORES  # 6250 cols per core
LN9 = float(np.log(9.0))

# worker tiles: 15 aligned tiles + one overlapping tail tile
_WSTARTS = [128 * t for t in range(15)] + [WORKERS - 128]

_CACHE = {}


def _build_nc():
    import concourse.bass as bass
    import concourse.mybir as mybir
    from concourse import bacc
    from concourse.tile import TileContext
    from contextlib import ExitStack

    f32 = mybir.dt.float32
    AF = mybir.ActivationFunctionType
    OP = mybir.AluOpType

    nc = bacc.Bacc("TRN2")
    wk = nc.dram_tensor("wk", [WORKERS, AB], f32, kind="ExternalInput")
    # this core's tau column slice, pre-replicated across 128 partitions
    tfb = nc.dram_tensor("tfb", [128, FS], f32, kind="ExternalInput")
    Wd = nc.dram_tensor("W", [AB, 1], f32, kind="ExternalInput")
    bd = nc.dram_tensor("b", [1], f32, kind="ExternalInput")
    out = nc.dram_tensor("out", [WORKERS, FS], f32, kind="ExternalOutput")

    NT = len(_WSTARTS)

    with TileContext(nc) as tc, ExitStack() as ctx:
        const = ctx.enter_context(tc.tile_pool(name="const", bufs=1))
        stage_p = ctx.enter_context(tc.tile_pool(name="stagep", bufs=3))

        # ---- per-worker scalars a (ACT scale) and c (ACT bias), all tiles
        Wb = const.tile([128, AB], f32, name="Wb")
        nc.sync.dma_start(
            out=Wb, in_=Wd[:].rearrange("a b -> b a").to_broadcast((128, AB))
        )
        bcol = const.tile([128, 1], f32, name="bcol")
        nc.sync.dma_start(out=bcol, in_=bd[:].to_broadcast((128, 1)))

        # ---- per-worker scalars in 4 batches of 4 tiles: lane p of column t
        # is worker 128*t+p (tail tile 15 starts at 1872 instead of 1920).
        # Batch 0 unblocks the first main ACT pair early; batching keeps the
        # activation count (and hence ACT table reloads) small.
        NB, TB = 4, NT // 4
        acol, ccol = [None] * NT, [None] * NT
        taub = const.tile([128, FS], f32, name="taub")
        for bi in range(NB):
            if bi == 1:
                # tau slice for this core: one balanced 16-engine DMA, issued
                # on the same sync ring AFTER batch 0's worker loads so those
                # small packets aren't starved behind tau's big ones
                nc.sync.dma_start(out=taub, in_=tfb[:])
            tlo = bi * TB
            wka = const.tile([128, TB, AB], f32, name=f"wka{bi}", tag="wka", bufs=2)
            if bi < NB - 1:
                srcb = wk[tlo * 128 : (tlo + TB) * 128, :].rearrange(
                    "(t p) a -> p t a", p=128
                )
                nc.sync.dma_start(out=wka, in_=srcb)
            else:
                srcb = wk[tlo * 128 : (tlo + TB - 1) * 128, :].rearrange(
                    "(t p) a -> p t a", p=128
                )
                nc.sync.dma_start(out=wka[:, 0 : TB - 1, :], in_=srcb)
                srct = wk[WORKERS - 128 : WORKERS, :].rearrange(
                    "(o p) a -> p o a", o=1
                )
                nc.sync.dma_start(out=wka[:, TB - 1 : TB, :], in_=srct)
            WbT = bass.AP(
                tensor=Wb.tensor,
                offset=Wb.offset,
                ap=[list(Wb.ap[0]), [0, TB], [1, AB]],
            )
            proda = const.tile(
                [128, TB, AB], f32, name=f"proda{bi}", tag="proda", bufs=2
            )
            nc.vector.tensor_mul(proda, wka, WbT)
            zb_ = const.tile([128, TB], f32, name=f"zb{bi}", tag="zb", bufs=2)
            nc.vector.reduce_sum(
                out=zb_.rearrange("p (t o) -> p t o", o=1),
                in_=proda,
                axis=mybir.AxisListType.X,
            )
            ab_ = const.tile([128, TB], f32, name=f"ab{bi}")
            nc.vector.tensor_scalar(
                out=ab_, in0=zb_, scalar1=bcol, scalar2=LN9, op0=OP.add, op1=OP.add
            )
            eb_ = const.tile([128, TB], f32, name=f"eb{bi}", tag="eb", bufs=2)
            nc.scalar.activation(out=eb_, in_=zb_, func=AF.Exp, bias=bcol, scale=1.0)
            lb_ = const.tile([128, TB], f32, name=f"lb{bi}", tag="lb", bufs=2)
            nc.scalar.activation(out=lb_, in_=eb_, func=AF.Ln, bias=1.0, scale=1.0)
            cb_ = const.tile([128, TB], f32, name=f"cb{bi}")
            nc.vector.tensor_scalar(
                out=cb_, in0=lb_, scalar1=-1.0, scalar2=-LN9, op0=OP.mult, op1=OP.add
            )
            for j in range(TB):
                acol[tlo + j] = ab_[:, j : j + 1]
                ccol[tlo + j] = cb_[:, j : j + 1]

        # ---- main loop: two worker tiles per staged store (fewer DMAs and
        # semaphore hops on the store ring); store is one 3D-AP DMA covering
        # 256 output rows
        for p in range(NT // 2):
            t0, t1 = 2 * p, 2 * p + 1
            stg = stage_p.tile([128, 2, FS], f32, name="stg", tag="stg")
            nc.scalar.activation(
                out=stg[:, 0, :],
                in_=taub,
                func=AF.Exp,
                bias=ccol[t0],
                scale=acol[t0],
            )
            nc.scalar.activation(
                out=stg[:, 1, :],
                in_=taub,
                func=AF.Exp,
                bias=ccol[t1],
                scale=acol[t1],
            )
            w0, w1 = _WSTARTS[t0], _WSTARTS[t1]
            if w1 == w0 + 128:
                dst = out[w0 : w0 + 256, :].rearrange("(c w) f -> w c f", c=2)
            else:
                # last pair overlaps (1792, 1872); build the two row-blocks
                # as an explicit strided AP on the row dimension
                dst = bass.AP(
                    tensor=out[:].tensor,
                    offset=w0 * FS,
                    ap=[[FS, 128], [(w1 - w0) * FS, 2], [1, FS]],
                )
            nc.sync.dma_start(out=dst, in_=stg)

    nc.compile()
    return nc


def _get_nc():
    if "nc" not in _CACHE:
        _CACHE["nc"] = _build_nc()
    return _CACHE["nc"]


def _make_in_maps(inputs_arr, W, b):
    wk = np.ascontiguousarray(inputs_arr[:WORKERS, :AB], dtype=np.float32)
    tau_flat = np.ascontiguousarray(
        inputs_arr[WORKERS:, :ET], dtype=np.float32
    ).reshape(F)
    W = np.ascontiguousarray(W, dtype=np.float32)
    b = np.ascontiguousarray(b, dtype=np.float32)
    maps = []
    for c in range(NCORES):
        tfb = np.ascontiguousarray(
            np.broadcast_to(tau_flat[c * FS : (c + 1) * FS], (128, FS))
        )
        maps.append({"wk": wk, "tfb": tfb, "W": W, "b": b})
    return maps


def _run(inputs_arr, W, b, **kwargs):
    from concourse import bass_utils

    nc = _get_nc()
    in_maps = _make_in_maps(inputs_arr, W, b)
    return bass_utils.run_bass_kernel_spmd(
        nc, in_maps, core_ids=list(range(NCORES)), **kwargs
    )


def kernel(inputs, W, b):
    inputs_arr = np.asarray(inputs, dtype=np.float32)
    res = _run(inputs_arr, np.asarray(W), np.asarray(b))
    out = np.concatenate([r["out"] for r in res.results], axis=1)
    return out.reshape(WORKERS, TASKS, ET)


# revision 26
# speedup vs baseline: 1.2530x; 1.0352x over previous
"""Trainium2 Bass kernel for nn_Decoder_36636071035490.

Reference computes, for workers i and task/edge (j,l):
    z = worker_feature @ W            # [2000, 1]
    p1 = sigmoid(z + b)
    p2 = (1 - p1) / 9
    P[i, j, l] = p1_i^tau_jl * p2_i^(1 - tau_jl)      # [2000, 5000, 10] f32

Identity used on device (exact in exact arithmetic):
    P[i, f] = exp(a_i * tau_f + c_i)
    a_i = (z_i + b) + ln 9            # since logit(sigmoid(x)) = x
    c_i = -ln(1 + exp(z_i + b)) - ln 9

Sharding: by output columns (task*edge flattened, 50000 -> 8 x 6250); every
core computes the cheap per-worker scalars a/c for all 2000 workers
(replicated matvec) and produces the full-height [2000, 6250] slab of P,
which is contiguous in its own output tensor. tau for the core's column
slice is passed pre-replicated across the 128 SBUF partitions ([128, 6250],
3.2 MB) so it loads as one balanced 16-engine DMA. The heavy math is a
single ScalarE ACTIVATE per 128-worker tile: out[p,f] = Exp(a_p*tau[f]+c_p)
via the activation's per-partition scale/bias operands. No PE, no PSUM.
All DMAs are full-128-partition (the SDMA splitter only uses all 16 engines
then); worker tile 15 overlaps tile 14 (rows 1872..1919 stored twice with
identical data) to keep 2000 = 15*128 + 80 full-width.
"""

import numpy as np

WORKERS = 2000
TASKS = 5000
ET = 10
AB = 64
NCORES = 8
F = TASKS * ET  # 50000 output cols
FS = F // NCORES  # 6250 cols per core
LN9 = float(np.log(9.0))

# worker tiles: 15 aligned tiles + one overlapping tail tile
_WSTARTS = [128 * t for t in range(15)] + [WORKERS - 128]

_CACHE = {}


def _build_nc():
    import concourse.bass as bass
    import concourse.mybir as mybir
    from concourse import bacc
    from concourse.tile import TileContext
    from contextlib import ExitStack

    f32 = mybir.dt.float32
    AF = mybir.ActivationFunctionType
    OP = mybir.AluOpType

    nc = bacc.Bacc("TRN2")
    wk = nc.dram_tensor("wk", [WORKERS, AB], f32, kind="ExternalInput")
    # this core's tau column slice, pre-replicated across 128 partitions
    tfb = nc.dram_tensor("tfb", [128, FS], f32, kind="ExternalInput")
    Wd = nc.dram_tensor("W", [AB, 1], f32, kind="ExternalInput")
    bd = nc.dram_tensor("b", [1], f32, kind="ExternalInput")
    out = nc.dram_tensor("out", [WORKERS, FS], f32, kind="ExternalOutput")

    NT = len(_WSTARTS)

    with TileContext(nc) as tc, ExitStack() as ctx:
        const = ctx.enter_context(tc.tile_pool(name="const", bufs=1))
        stage_p = ctx.enter_context(tc.tile_pool(name="stagep", bufs=3))

        # ---- per-worker scalars a (ACT scale) and c (ACT bias), all tiles
        Wb = const.tile([128, AB], f32, name="Wb")
        nc.sync.dma_start(
            out=Wb, in_=Wd[:].rearrange("a b -> b a").to_broadcast((128, AB))
        )
        bcol = const.tile([128, 1], f32, name="bcol")
        nc.sync.dma_start(out=bcol, in_=bd[:].to_broadcast((128, 1)))

        # ---- per-worker scalars in 4 batches of 4 tiles: lane p of column t
        # is worker 128*t+p (tail tile 15 starts at 1872 instead of 1920).
        # Batch 0 unblocks the first main ACT pair early; batching keeps the
        # activation count (and hence ACT table reloads) small.
        NB, TB = 2, NT // 2
        acol, ccol = [None] * NT, [None] * NT
        taub = const.tile([128, FS], f32, name="taub")
        wkab = []
        for bi in range(NB):
            tlo = bi * TB
            wka = const.tile([128, TB, AB], f32, name=f"wka{bi}", tag=f"wka{bi}")
            wkab.append(wka)
            if bi < NB - 1:
                srcb = wk[tlo * 128 : (tlo + TB) * 128, :].rearrange(
                    "(t p) a -> p t a", p=128
                )
                nc.sync.dma_start(out=wka, in_=srcb)
            else:
                srcb = wk[tlo * 128 : (tlo + TB - 1) * 128, :].rearrange(
                    "(t p) a -> p t a", p=128
                )
                nc.sync.dma_start(out=wka[:, 0 : TB - 1, :], in_=srcb)
                srct = wk[WORKERS - 128 : WORKERS, :].rearrange(
                    "(o p) a -> p o a", o=1
                )
                nc.sync.dma_start(out=wka[:, TB - 1 : TB, :], in_=srct)
        # tau slice for this core: one balanced 16-engine DMA on the same sync
        # ring AFTER the (small-packet) worker loads so they aren't starved
        nc.sync.dma_start(out=taub, in_=tfb[:])
        for bi in range(NB):
            tlo = bi * TB
            wka = wkab[bi]
            WbT = bass.AP(
                tensor=Wb.tensor,
                offset=Wb.offset,
                ap=[list(Wb.ap[0]), [0, TB], [1, AB]],
            )
            proda = const.tile(
                [128, TB, AB], f32, name=f"proda{bi}", tag="proda", bufs=2
            )
            nc.vector.tensor_mul(proda, wka, WbT)
            zb_ = const.tile([128, TB], f32, name=f"zb{bi}", tag="zb", bufs=2)
            nc.vector.reduce_sum(
                out=zb_.rearrange("p (t o) -> p t o", o=1),
                in_=proda,
                axis=mybir.AxisListType.X,
            )
            ab_ = const.tile([128, TB], f32, name=f"ab{bi}")
            nc.vector.tensor_scalar(
                out=ab_, in0=zb_, scalar1=bcol, scalar2=LN9, op0=OP.add, op1=OP.add
            )
            eb_ = const.tile([128, TB], f32, name=f"eb{bi}", tag="eb", bufs=2)
            nc.scalar.activation(out=eb_, in_=zb_, func=AF.Exp, bias=bcol, scale=1.0)
            lb_ = const.tile([128, TB], f32, name=f"lb{bi}", tag="lb", bufs=2)
            nc.scalar.activation(out=lb_, in_=eb_, func=AF.Ln, bias=1.0, scale=1.0)
            cb_ = const.tile([128, TB], f32, name=f"cb{bi}")
            nc.vector.tensor_scalar(
                out=cb_, in0=lb_, scalar1=-1.0, scalar2=-LN9, op0=OP.mult, op1=OP.add
            )
            for j in range(TB):
                acol[tlo + j] = ab_[:, j : j + 1]
                ccol[tlo + j] = cb_[:, j : j + 1]

        # ---- main loop. First two tiles store individually so the store
        # stream starts as soon as one ACT is done; remaining pairs share a
        # double-width stage and store via one 3D-AP DMA covering 256 rows
        # (fewer DMAs and semaphore hops on the store ring).
        for t in (0, 1):
            stg1 = stage_p.tile([128, 2, FS], f32, name=f"stg1_{t}", tag="stg")
            nc.scalar.activation(
                out=stg1[:, 0, :],
                in_=taub,
                func=AF.Exp,
                bias=ccol[t],
                scale=acol[t],
            )
            w0 = _WSTARTS[t]
            nc.sync.dma_start(out=out[w0 : w0 + 128, :], in_=stg1[:, 0, :])
        for p in range(1, NT // 2):
            t0, t1 = 2 * p, 2 * p + 1
            stg = stage_p.tile([128, 2, FS], f32, name="stg", tag="stg")
            nc.scalar.activation(
                out=stg[:, 0, :],
                in_=taub,
                func=AF.Exp,
                bias=ccol[t0],
                scale=acol[t0],
            )
            nc.scalar.activation(
                out=stg[:, 1, :],
                in_=taub,
                func=AF.Exp,
                bias=ccol[t1],
                scale=acol[t1],
            )
            w0, w1 = _WSTARTS[t0], _WSTARTS[t1]
            if w1 == w0 + 128:
                dst = out[w0 : w0 + 256, :].rearrange("(c w) f -> w c f", c=2)
            else:
                # last pair overlaps (1792, 1872); build the two row-blocks
                # as an explicit strided AP on the row dimension
                dst = bass.AP(
                    tensor=out[:].tensor,
                    offset=w0 * FS,
                    ap=[[FS, 128], [(w1 - w0) * FS, 2], [1, FS]],
                )
            nc.sync.dma_start(out=dst, in_=stg)

    nc.compile()
    return nc


def _get_nc():
    if "nc" not in _CACHE:
        _CACHE["nc"] = _build_nc()
    return _CACHE["nc"]


def _make_in_maps(inputs_arr, W, b):
    wk = np.ascontiguousarray(inputs_arr[:WORKERS, :AB], dtype=np.float32)
    tau_flat = np.ascontiguousarray(
        inputs_arr[WORKERS:, :ET], dtype=np.float32
    ).reshape(F)
    W = np.ascontiguousarray(W, dtype=np.float32)
    b = np.ascontiguousarray(b, dtype=np.float32)
    maps = []
    for c in range(NCORES):
        tfb = np.ascontiguousarray(
            np.broadcast_to(tau_flat[c * FS : (c + 1) * FS], (128, FS))
        )
        maps.append({"wk": wk, "tfb": tfb, "W": W, "b": b})
    return maps


def _run(inputs_arr, W, b, **kwargs):
    from concourse import bass_utils

    nc = _get_nc()
    in_maps = _make_in_maps(inputs_arr, W, b)
    return bass_utils.run_bass_kernel_spmd(
        nc, in_maps, core_ids=list(range(NCORES)), **kwargs
    )


def kernel(inputs, W, b):
    inputs_arr = np.asarray(inputs, dtype=np.float32)
    res = _run(inputs_arr, np.asarray(W), np.asarray(b))
    out = np.concatenate([r["out"] for r in res.results], axis=1)
    return out.reshape(WORKERS, TASKS, ET)


# revision 28
# speedup vs baseline: 1.2768x; 1.0190x over previous
"""Trainium2 Bass kernel for nn_Decoder_36636071035490.

Reference computes, for workers i and task/edge (j,l):
    z = worker_feature @ W            # [2000, 1]
    p1 = sigmoid(z + b)
    p2 = (1 - p1) / 9
    P[i, j, l] = p1_i^tau_jl * p2_i^(1 - tau_jl)      # [2000, 5000, 10] f32

Identity used on device (exact in exact arithmetic):
    P[i, f] = exp(a_i * tau_f + c_i)
    a_i = (z_i + b) + ln 9            # since logit(sigmoid(x)) = x
    c_i = -ln(1 + exp(z_i + b)) - ln 9

Sharding: by output columns (task*edge flattened, 50000 -> 8 x 6250); every
core computes the cheap per-worker scalars a/c for all 2000 workers
(replicated matvec) and produces the full-height [2000, 6250] slab of P,
which is contiguous in its own output tensor. tau for the core's column
slice is passed pre-replicated across the 128 SBUF partitions ([128, 6250],
3.2 MB) so it loads as one balanced 16-engine DMA. The heavy math is a
single ScalarE ACTIVATE per 128-worker tile: out[p,f] = Exp(a_p*tau[f]+c_p)
via the activation's per-partition scale/bias operands. No PE, no PSUM.
All DMAs are full-128-partition (the SDMA splitter only uses all 16 engines
then); worker tile 15 overlaps tile 14 (rows 1872..1919 stored twice with
identical data) to keep 2000 = 15*128 + 80 full-width.
"""

import numpy as np

WORKERS = 2000
TASKS = 5000
ET = 10
AB = 64
NCORES = 8
F = TASKS * ET  # 50000 output cols
FS = F // NCORES  # 6250 cols per core
LN9 = float(np.log(9.0))

# worker tiles: 15 aligned tiles + one overlapping tail tile
_WSTARTS = [128 * t for t in range(15)] + [WORKERS - 128]

_CACHE = {}


def _build_nc():
    import concourse.bass as bass
    import concourse.mybir as mybir
    from concourse import bacc
    from concourse.tile import TileContext
    from contextlib import ExitStack

    f32 = mybir.dt.float32
    AF = mybir.ActivationFunctionType
    OP = mybir.AluOpType

    nc = bacc.Bacc("TRN2")
    wk = nc.dram_tensor("wk", [WORKERS, AB], f32, kind="ExternalInput")
    # this core's tau column slice, pre-replicated across 128 partitions
    tfb = nc.dram_tensor("tfb", [128, FS], f32, kind="ExternalInput")
    Wd = nc.dram_tensor("W", [AB, 1], f32, kind="ExternalInput")
    bd = nc.dram_tensor("b", [1], f32, kind="ExternalInput")
    out = nc.dram_tensor("out", [WORKERS, FS], f32, kind="ExternalOutput")

    NT = len(_WSTARTS)

    with TileContext(nc) as tc, ExitStack() as ctx:
        const = ctx.enter_context(tc.tile_pool(name="const", bufs=1))
        stage_p = ctx.enter_context(tc.tile_pool(name="stagep", bufs=3))

        # ---- per-worker scalars a (ACT scale) and c (ACT bias), all tiles
        Wb = const.tile([128, AB], f32, name="Wb")
        nc.sync.dma_start(
            out=Wb, in_=Wd[:].rearrange("a b -> b a").to_broadcast((128, AB))
        )
        bcol = const.tile([128, 1], f32, name="bcol")
        nc.sync.dma_start(out=bcol, in_=bd[:].to_broadcast((128, 1)))

        # ---- per-worker scalars in 4 batches of 4 tiles: lane p of column t
        # is worker 128*t+p (tail tile 15 starts at 1872 instead of 1920).
        # Batch 0 unblocks the first main ACT pair early; batching keeps the
        # activation count (and hence ACT table reloads) small.
        NB, TB = 2, NT // 2
        acol, ccol = [None] * NT, [None] * NT
        taub = const.tile([128, FS], f32, name="taub")
        wkab = []
        for bi in range(NB):
            tlo = bi * TB
            wka = const.tile([128, TB, AB], f32, name=f"wka{bi}", tag=f"wka{bi}")
            wkab.append(wka)
            if bi < NB - 1:
                srcb = wk[tlo * 128 : (tlo + TB) * 128, :].rearrange(
                    "(t p) a -> p t a", p=128
                )
                nc.sync.dma_start(out=wka, in_=srcb)
            else:
                srcb = wk[tlo * 128 : (tlo + TB - 1) * 128, :].rearrange(
                    "(t p) a -> p t a", p=128
                )
                nc.sync.dma_start(out=wka[:, 0 : TB - 1, :], in_=srcb)
                srct = wk[WORKERS - 128 : WORKERS, :].rearrange(
                    "(o p) a -> p o a", o=1
                )
                nc.sync.dma_start(out=wka[:, TB - 1 : TB, :], in_=srct)
        # tau slice for this core: two balanced 16-engine DMAs on the same
        # sync ring AFTER the (small-packet) worker loads so they aren't
        # starved; halved so the first ACTs can start on the first half
        FH = FS // 2
        nc.sync.dma_start(out=taub[:, 0:FH], in_=tfb[:, 0:FH])
        nc.sync.dma_start(out=taub[:, FH:FS], in_=tfb[:, FH:FS])
        for bi in range(NB):
            tlo = bi * TB
            wka = wkab[bi]
            WbT = bass.AP(
                tensor=Wb.tensor,
                offset=Wb.offset,
                ap=[list(Wb.ap[0]), [0, TB], [1, AB]],
            )
            proda = const.tile(
                [128, TB, AB], f32, name=f"proda{bi}", tag="proda", bufs=2
            )
            nc.vector.tensor_mul(proda, wka, WbT)
            zb_ = const.tile([128, TB], f32, name=f"zb{bi}", tag="zb", bufs=2)
            nc.vector.reduce_sum(
                out=zb_.rearrange("p (t o) -> p t o", o=1),
                in_=proda,
                axis=mybir.AxisListType.X,
            )
            ab_ = const.tile([128, TB], f32, name=f"ab{bi}")
            nc.vector.tensor_scalar(
                out=ab_, in0=zb_, scalar1=bcol, scalar2=LN9, op0=OP.add, op1=OP.add
            )
            eb_ = const.tile([128, TB], f32, name=f"eb{bi}", tag="eb", bufs=2)
            nc.scalar.activation(out=eb_, in_=zb_, func=AF.Exp, bias=bcol, scale=1.0)
            lb_ = const.tile([128, TB], f32, name=f"lb{bi}", tag="lb", bufs=2)
            nc.scalar.activation(out=lb_, in_=eb_, func=AF.Ln, bias=1.0, scale=1.0)
            cb_ = const.tile([128, TB], f32, name=f"cb{bi}")
            nc.vector.tensor_scalar(
                out=cb_, in0=lb_, scalar1=-1.0, scalar2=-LN9, op0=OP.mult, op1=OP.add
            )
            for j in range(TB):
                acol[tlo + j] = ab_[:, j : j + 1]
                ccol[tlo + j] = cb_[:, j : j + 1]

        # ---- main loop. First two tiles store individually so the store
        # stream starts as soon as one ACT is done; remaining pairs share a
        # double-width stage and store via one 3D-AP DMA covering 256 rows
        # (fewer DMAs and semaphore hops on the store ring).
        for t in (0, 1):
            stg1 = stage_p.tile([128, 2, FS], f32, name=f"stg1_{t}", tag="stg")
            w0 = _WSTARTS[t]
            for h, (c0, c1) in enumerate(((0, FH), (FH, FS))):
                nc.scalar.activation(
                    out=stg1[:, 0, c0:c1],
                    in_=taub[:, c0:c1],
                    func=AF.Exp,
                    bias=ccol[t],
                    scale=acol[t],
                )
                nc.sync.dma_start(
                    out=out[w0 : w0 + 128, c0:c1], in_=stg1[:, 0, c0:c1]
                )
        for p in range(1, NT // 2):
            t0, t1 = 2 * p, 2 * p + 1
            stg = stage_p.tile([128, 2, FS], f32, name="stg", tag="stg")
            nc.scalar.activation(
                out=stg[:, 0, :],
                in_=taub,
                func=AF.Exp,
                bias=ccol[t0],
                scale=acol[t0],
            )
            nc.scalar.activation(
                out=stg[:, 1, :],
                in_=taub,
                func=AF.Exp,
                bias=ccol[t1],
                scale=acol[t1],
            )
            w0, w1 = _WSTARTS[t0], _WSTARTS[t1]
            if w1 == w0 + 128:
                dst = out[w0 : w0 + 256, :].rearrange("(c w) f -> w c f", c=2)
            else:
                # last pair overlaps (1792, 1872); build the two row-blocks
                # as an explicit strided AP on the row dimension
                dst = bass.AP(
                    tensor=out[:].tensor,
                    offset=w0 * FS,
                    ap=[[FS, 128], [(w1 - w0) * FS, 2], [1, FS]],
                )
            nc.sync.dma_start(out=dst, in_=stg)

    nc.compile()
    return nc


def _get_nc():
    if "nc" not in _CACHE:
        _CACHE["nc"] = _build_nc()
    return _CACHE["nc"]


def _make_in_maps(inputs_arr, W, b):
    wk = np.ascontiguousarray(inputs_arr[:WORKERS, :AB], dtype=np.float32)
    tau_flat = np.ascontiguousarray(
        inputs_arr[WORKERS:, :ET], dtype=np.float32
    ).reshape(F)
    W = np.ascontiguousarray(W, dtype=np.float32)
    b = np.ascontiguousarray(b, dtype=np.float32)
    maps = []
    for c in range(NCORES):
        tfb = np.ascontiguousarray(
            np.broadcast_to(tau_flat[c * FS : (c + 1) * FS], (128, FS))
        )
        maps.append({"wk": wk, "tfb": tfb, "W": W, "b": b})
    return maps


def _run(inputs_arr, W, b, **kwargs):
    from concourse import bass_utils

    nc = _get_nc()
    in_maps = _make_in_maps(inputs_arr, W, b)
    return bass_utils.run_bass_kernel_spmd(
        nc, in_maps, core_ids=list(range(NCORES)), **kwargs
    )


def kernel(inputs, W, b):
    inputs_arr = np.asarray(inputs, dtype=np.float32)
    res = _run(inputs_arr, np.asarray(W), np.asarray(b))
    out = np.concatenate([r["out"] for r in res.results], axis=1)
    return out.reshape(WORKERS, TASKS, ET)
